# revision 13
# baseline (speedup 1.0000x reference)
"""Cross-attention kernel for Trainium2 (8 NeuronCores, SPMD).

Problem: B=4, LQ=LK=4096, H=256
  query = q @ Wq.T + bq ; keys = k @ Wk.T + bk ; values = v @ Wv.T + bv
  out = softmax(query @ keys.T / sqrt(H)) @ values

Sharding: core i -> batch i//2, query rows (i%2)*2048 .. +2048.
K/V for the batch are replicated across the 2 cores sharing it.

Device algorithm (PE contracts over the partition dim):
  - scores are algebraically refactored:
      s[q,k] = q_q M k_k^T + t_q + u_k,  M = Wq.T @ Wk  (host-folded)
      t_q = (q Wq.T)·bk   -- constant per softmax row: cancels, dropped
      u_k = (k·(Wk.T bq) + bq·bk)/sqrt(H) -- per-key scalar, computed on
            host during input prep, folded into exp as per-partition bias
    so the K projection disappears and scores read RAW k^T.
  - q/k/v are fed transposed ([h, s], h on partitions); scores are
    computed transposed ([k, q]) so exp(scores) = P^T is born k-major.
  - softmax skips max-subtraction (scores/sqrt(H) stay within ~+-7 here).
  - P@V uses P^T tiles as stationary and V augmented with a ones-column
    ([k, 257]) as moving: output column 256 is the softmax denominator
    and the context lands in natural [q, h] layout. Normalization is a
    per-partition reciprocal + tensor_scalar multiply on PSUM->SBUF.
  - score and P@V matmuls are interleaved per k-tile (P@V lags 4 tiles)
    so the exp's ScalarE latency hides behind P@V work on PE; the V and
    qM projections fill the first chunk's score phase.
"""

import os
import sys

import numpy as np

sys.path.insert(0, "/opt/trn_rl_repo")

import ml_dtypes

B, LQ, LK, H = 4, 4096, 4096, 256
P = 128
HO = H // P            # 2 h-tiles
NCORES = 8
NQ = LQ * B // NCORES  # 2048 q rows per core
QC = 512               # q chunk (scores tile width)
NQC = NQ // QC         # 4
QW = QC // P           # 4 q-windows per chunk
KT = LK // P           # 32 k tiles
HA = H + 1             # V augmented with ones column
LAG = 4                # P@V lags scores by this many k-tiles
SCALE = 1.0 / np.sqrt(np.float32(H))  # 1/16

_BF16 = ml_dtypes.bfloat16

_NC_CACHE = None


def _build_nc():
    """Build the single-core Bass program (same program runs SPMD on 8 cores)."""
    import concourse.bass as bass
    import concourse.mybir as mybir
    import concourse.tile as tile
    from concourse import bacc

    f32 = mybir.dt.float32
    bf16 = mybir.dt.bfloat16

    nc = bacc.Bacc("TRN2", target_bir_lowering=False, debug=False)

    kT = nc.declare_dram_parameter("kT", [H, LK], bf16, isOutput=False)
    qT = nc.declare_dram_parameter("qT", [H, NQ], bf16, isOutput=False)
    vT = nc.declare_dram_parameter("vT", [H, LK], bf16, isOutput=False)
    mT = nc.declare_dram_parameter("mT", [H, H], bf16, isOutput=False)   # M=Wq.T@Wk
    wvT = nc.declare_dram_parameter("wvT", [H, H], bf16, isOutput=False)
    ub = nc.declare_dram_parameter("ub", [P, KT], f32, isOutput=False)   # exp bias
    bvr = nc.declare_dram_parameter("bvr", [P, H], f32, isOutput=False)
    out = nc.declare_dram_parameter("out", [NQ, H], f32, isOutput=True)

    # [h, s] -> [p, ho, s] with h = ho*128 + p
    qT_r = qT.ap().rearrange("(o p) n -> p o n", p=P)
    kT_r = kT.ap().rearrange("(o p) n -> p o n", p=P)
    vT_r = vT.ap().rearrange("(o p) n -> p o n", p=P)
    m_r = mT.ap().rearrange("(o p) n -> p o n", p=P)
    wv_r = wvT.ap().rearrange("(o p) n -> p o n", p=P)

    Exp = mybir.ActivationFunctionType.Exp
    Add = mybir.AluOpType.add

    with tile.TileContext(nc) as tc:
        with (
            tc.tile_pool(name="consts", bufs=1) as consts,
            tc.tile_pool(name="persist", bufs=1) as persist,
        ):
            m_sb = consts.tile([P, HO, H], bf16)
            wv_sb = consts.tile([P, HO, H], bf16)
            u_sb = consts.tile([P, KT], f32)
            bv_sb = consts.tile([P, H], f32)

            kraw = persist.tile([P, HO, LK], bf16)
            qraw = persist.tile([P, HO, NQ], bf16)
            vraw = persist.tile([P, HO, LK], bf16)
            QMT = persist.tile([P, HO, NQ], bf16)   # (q M)^T  [h~, q]
            V_sb = persist.tile([P, KT, HA], bf16)  # values [k, h] + ones col

            # DMA issue order = consumption order, split across the sync and
            # gpsimd engines so per-instruction issue time (~0.8us) doesn't
            # serialize the startup: sync takes M + k chunks (needed first and
            # progressively), gpsimd takes q + v + the remaining weights.
            nc.sync.dma_start(m_sb[:], m_r)
            nc.sync.dma_start(u_sb[:], ub.ap())
            nc.gpsimd.dma_start(qraw[:, :, :QC], qT_r[:, :, :QC])
            nc.gpsimd.dma_start(wv_sb[:], wv_r)
            nc.gpsimd.dma_start(bv_sb[:], bvr.ap())
            KCH = LK // 4
            for c in range(4):
                sl = slice(c * KCH, (c + 1) * KCH)
                nc.sync.dma_start(kraw[:, :, sl], kT_r[:, :, sl])
                nc.gpsimd.dma_start(vraw[:, :, sl], vT_r[:, :, sl])
            nc.gpsimd.dma_start(qraw[:, :, QC:], qT_r[:, :, QC:])
            nc.vector.memset(V_sb[:, :, H:HA], 1.0)

            with (
                tc.tile_pool(name="pproj", bufs=2, space="PSUM") as pp,
                tc.tile_pool(name="pt", bufs=10) as ptp,
                tc.tile_pool(name="ps_s", bufs=2, space="PSUM") as pss,
                tc.tile_pool(name="ps_ctx", bufs=4, space="PSUM") as psc,
                tc.tile_pool(name="fin", bufs=4) as fin,
            ):
                # (qM)^T projection chunk: lhsT = M[h, h~-window], rhs = qraw
                def qm_chunk(c):
                    for ot in range(HO):
                        ps = pp.tile([P, QC], f32, tag="pp")
                        for ho in range(HO):
                            nc.tensor.matmul(
                                ps[:],
                                m_sb[:, ho, ot * P:(ot + 1) * P],
                                qraw[:, ho, c * QC:(c + 1) * QC],
                                start=(ho == 0),
                                stop=(ho == HO - 1),
                            )
                        nc.vector.tensor_copy(
                            QMT[:, ot, c * QC:(c + 1) * QC], ps[:]
                        )

                # V projection chunk: V[s, o] = vraw-tile.T @ Wv^T + bv
                def v_chunk(st):
                    ps_full = pp.tile([P, QC], f32, tag="pp")
                    ps = ps_full[:, :H]
                    for ho in range(HO):
                        nc.tensor.matmul(
                            ps[:],
                            vraw[:, ho, st * P:(st + 1) * P],
                            wv_sb[:, ho, :],
                            start=(ho == 0),
                            stop=(ho == HO - 1),
                        )
                    nc.vector.tensor_tensor(V_sb[:, st, :H], ps[:], bv_sb[:], Add)

                def scores_tile(qc, kt, pts):
                    ps = pss.tile([P, QC], f32, tag="ps_s")
                    for ho in range(HO):
                        nc.tensor.matmul(
                            ps[:],
                            kraw[:, ho, kt * P:(kt + 1) * P],
                            QMT[:, ho, qc * QC:(qc + 1) * QC],
                            start=(ho == 0),
                            stop=(ho == HO - 1),
                        )
                    pt = ptp.tile([P, QC], bf16, tag="pt")
                    nc.scalar.activation(
                        pt[:], ps[:], Exp,
                        bias=u_sb[:, kt:kt + 1], scale=float(SCALE),
                    )
                    pts[kt] = pt

                def pv_step(ctx, kt, pts):
                    for qw in range(QW):
                        nc.tensor.matmul(
                            ctx[qw][:],
                            pts[kt][:, qw * P:(qw + 1) * P],
                            V_sb[:, kt, :],
                            start=(kt == 0),
                            stop=(kt == KT - 1),
                        )

                qm_chunk(0)
                for qc in range(NQC):
                    ctx = [psc.tile([P, HA], f32, tag="ps_ctx",
                                    name=f"ctx_{qc}_{qw}")
                           for qw in range(QW)]
                    pts = {}
                    for kt in range(KT):
                        scores_tile(qc, kt, pts)
                        if qc == 0:
                            # fill the first chunk's exp-bound phase with
                            # the V projection and remaining qM chunks
                            v_chunk(kt)
                            if kt % 12 == 4 and 1 + kt // 12 < NQC:
                                qm_chunk(1 + kt // 12)
                        if kt >= LAG:
                            pv_step(ctx, kt - LAG, pts)
                    for kt in range(KT - LAG, KT):
                        pv_step(ctx, kt, pts)
                    for qw in range(QW):
                        rec = fin.tile([P, 1], f32, tag="rec")
                        nc.vector.reciprocal(rec[:], ctx[qw][:, H:HA])
                        osb = fin.tile([P, H], f32, tag="osb")
                        nc.vector.tensor_scalar_mul(
                            osb[:], ctx[qw][:, :H], rec[:])
                        nc.sync.dma_start(
                            out.ap()[qc * QC + qw * P:qc * QC + (qw + 1) * P, :],
                            osb[:],
                        )
    nc.compile()
    return nc


def _get_nc():
    global _NC_CACHE
    if _NC_CACHE is None:
        _NC_CACHE = _build_nc()
    return _NC_CACHE


def _prep_in_maps(q, k, v, Wq, bq, Wk, bk, Wv, bv):
    q = np.asarray(q, np.float32)
    k = np.asarray(k, np.float32)
    v = np.asarray(v, np.float32)
    Wq = np.asarray(Wq, np.float64)
    Wk = np.asarray(Wk, np.float64)
    bq_ = np.asarray(bq, np.float64)
    bk_ = np.asarray(bk, np.float64)
    M = Wq.T @ Wk                       # [h, h~]
    w2v = Wk.T @ bq_                    # [h]
    ccv = float(bq_ @ bk_)
    mT = np.ascontiguousarray(M).astype(_BF16)          # [h, h~] == lhsT layout
    wvT = np.ascontiguousarray(np.asarray(Wv, np.float32).T).astype(_BF16)
    bvr = np.ascontiguousarray(
        np.broadcast_to(np.asarray(bv, np.float32), (P, H)))
    in_maps = []
    for i in range(NCORES):
        b, half = divmod(i, NCORES // B)
        qT_i = np.ascontiguousarray(q[b, half * NQ:(half + 1) * NQ, :].T).astype(_BF16)
        kT_i = np.ascontiguousarray(k[b].T).astype(_BF16)
        vT_i = np.ascontiguousarray(v[b].T).astype(_BF16)
        # u_k = (k.(Wk.T bq) + bq.bk)/sqrt(H), [k] -> [p, kt] with k=kt*128+p
        u = (k[b].astype(np.float64) @ w2v + ccv) * float(SCALE)
        ub_i = np.ascontiguousarray(u.reshape(KT, P).T.astype(np.float32))
        in_maps.append({
            "qT": qT_i, "kT": kT_i, "vT": vT_i,
            "mT": mT, "wvT": wvT, "ub": ub_i, "bvr": bvr,
        })
    return in_maps


def _install_ntff_hook_shim():
    """The image's antenv lacks axon_hooks; recreate it from the boot recipe
    (ctypes into libaxon_pjrt.so) so trace=True can capture NTFF profiles."""
    import types
    import contextlib
    import ctypes

    if "antenv.axon_hooks" in sys.modules:
        return
    so_path = "/opt/axon/libaxon_pjrt.so"
    hook = None
    if os.path.exists(so_path):
        lib = ctypes.CDLL(so_path)
        if hasattr(lib, "axon_start_nrt_profile"):
            lib.axon_start_nrt_profile.argtypes = [
                ctypes.POINTER(ctypes.c_int64), ctypes.c_size_t]
            lib.axon_start_nrt_profile.restype = ctypes.c_int64
            lib.axon_stop_nrt_profile.argtypes = [ctypes.c_char_p]
            lib.axon_stop_nrt_profile.restype = ctypes.c_int64

            @contextlib.contextmanager
            def _hook(output_dir, device_ids):
                import jax
                jax.devices()
                if device_ids:
                    ids = (ctypes.c_int64 * len(device_ids))(*device_ids)
                    rc = lib.axon_start_nrt_profile(ids, len(device_ids))
                else:
                    rc = lib.axon_start_nrt_profile(None, 0)
                if rc != 0:
                    raise RuntimeError(f"axon_start_nrt_profile rc={rc}")
                try:
                    yield
                finally:
                    n = lib.axon_stop_nrt_profile(str(output_dir).encode())
                    print(f"profile: {n} file(s) written to {output_dir}")

            hook = _hook
    mod = types.ModuleType("antenv.axon_hooks")
    mod.get_axon_ntff_profile_hook = lambda: hook
    mod.set_axon_ntff_profile_hook = lambda h: None
    sys.modules["antenv.axon_hooks"] = mod


def run(inputs, trace=False, trace_cores=None):
    """Run on 8 NeuronCores. Returns (output, BassKernelResults)."""
    from concourse.bass_utils import run_bass_kernel_spmd

    if trace:
        _install_ntff_hook_shim()
    nc = _get_nc()
    in_maps = _prep_in_maps(**inputs)
    res = run_bass_kernel_spmd(
        nc, in_maps, core_ids=list(range(NCORES)),
        trace=trace, trace_cores=trace_cores,
    )
    full = np.empty((B, LQ, H), np.float32)
    for i in range(NCORES):
        b, half = divmod(i, NCORES // B)
        full[b, half * NQ:(half + 1) * NQ, :] = res.results[i]["out"]
    return full, res


def kernel(**inputs):
    return run(inputs, trace=False)[0]


# revision 14
# speedup vs baseline: 1.0362x; 1.0362x over previous
"""Cross-attention kernel for Trainium2 (8 NeuronCores, SPMD).

Problem: B=4, LQ=LK=4096, H=256
  query = q @ Wq.T + bq ; keys = k @ Wk.T + bk ; values = v @ Wv.T + bv
  out = softmax(query @ keys.T / sqrt(H)) @ values

Sharding: core i -> batch i//2, query rows (i%2)*2048 .. +2048.
K/V for the batch are replicated across the 2 cores sharing it.

Device algorithm (PE contracts over the partition dim):
  - scores are algebraically refactored:
      s[q,k] = q_q M k_k^T + t_q + u_k,  M = Wq.T @ Wk  (host-folded)
      t_q = (q Wq.T)·bk   -- constant per softmax row: cancels, dropped
      u_k = (k·(Wk.T bq) + bq·bk)/sqrt(H) -- per-key scalar, computed on
            host during input prep, folded into exp as per-partition bias
    so the K projection disappears and scores read RAW k^T.
  - q/k/v are fed transposed ([h, s], h on partitions); scores are
    computed transposed ([k, q]) so exp(scores) = P^T is born k-major.
  - softmax skips max-subtraction (scores/sqrt(H) stay within ~+-7 here).
  - P@V uses P^T tiles as stationary and V augmented with a ones-column
    ([k, 257]) as moving: output column 256 is the softmax denominator
    and the context lands in natural [q, h] layout. Normalization is a
    per-partition reciprocal + tensor_scalar multiply on PSUM->SBUF.
  - score and P@V matmuls are interleaved per k-tile (P@V lags 4 tiles)
    so the exp's ScalarE latency hides behind P@V work on PE; the V and
    qM projections fill the first chunk's score phase.
"""

import os
import sys

import numpy as np

sys.path.insert(0, "/opt/trn_rl_repo")

import ml_dtypes

B, LQ, LK, H = 4, 4096, 4096, 256
P = 128
HO = H // P            # 2 h-tiles
NCORES = 8
NQ = LQ * B // NCORES  # 2048 q rows per core
QC = 512               # q chunk (scores tile width)
NQC = NQ // QC         # 4
QW = QC // P           # 4 q-windows per chunk
KT = LK // P           # 32 k tiles
HA = H + 1             # V augmented with ones column
LAG = 4                # P@V lags scores by this many k-tiles
SCALE = 1.0 / np.sqrt(np.float32(H))  # 1/16

_BF16 = ml_dtypes.bfloat16

_NC_CACHE = None


def _build_nc():
    """Build the single-core Bass program (same program runs SPMD on 8 cores)."""
    import concourse.bass as bass
    import concourse.mybir as mybir
    import concourse.tile as tile
    from concourse import bacc

    f32 = mybir.dt.float32
    bf16 = mybir.dt.bfloat16

    nc = bacc.Bacc("TRN2", target_bir_lowering=False, debug=False)

    kT = nc.declare_dram_parameter("kT", [H, LK], bf16, isOutput=False)
    qT = nc.declare_dram_parameter("qT", [H, NQ], bf16, isOutput=False)
    vT = nc.declare_dram_parameter("vT", [H, LK], bf16, isOutput=False)
    mT = nc.declare_dram_parameter("mT", [H, H], bf16, isOutput=False)   # M=Wq.T@Wk
    wvT = nc.declare_dram_parameter("wvT", [H, H], bf16, isOutput=False)
    ub = nc.declare_dram_parameter("ub", [P, KT], f32, isOutput=False)   # exp bias
    bvr = nc.declare_dram_parameter("bvr", [P, H], f32, isOutput=False)
    out = nc.declare_dram_parameter("out", [NQ, H], f32, isOutput=True)

    # [h, s] -> [p, ho, s] with h = ho*128 + p
    qT_r = qT.ap().rearrange("(o p) n -> p o n", p=P)
    kT_r = kT.ap().rearrange("(o p) n -> p o n", p=P)
    vT_r = vT.ap().rearrange("(o p) n -> p o n", p=P)
    m_r = mT.ap().rearrange("(o p) n -> p o n", p=P)
    wv_r = wvT.ap().rearrange("(o p) n -> p o n", p=P)

    Exp = mybir.ActivationFunctionType.Exp
    Add = mybir.AluOpType.add

    with tile.TileContext(nc) as tc:
        with (
            tc.tile_pool(name="consts", bufs=1) as consts,
            tc.tile_pool(name="persist", bufs=1) as persist,
        ):
            m_sb = consts.tile([P, HO, H], bf16)
            wv_sb = consts.tile([P, HO, H], bf16)
            u_sb = consts.tile([P, KT], f32)
            bv_sb = consts.tile([P, H], f32)

            kraw = persist.tile([P, HO, LK], bf16)
            qraw = persist.tile([P, HO, NQ], bf16)
            vraw = persist.tile([P, HO, LK], bf16)
            QMT = persist.tile([P, HO, NQ], bf16)   # (q M)^T  [h~, q]
            V_sb = persist.tile([P, KT, HA], bf16)  # values [k, h] + ones col

            # DMA issue order = consumption order. Small weight tensors on
            # the sync engine; bulk k/q/v loads issued from the otherwise-idle
            # gpsimd engine so issue time doesn't serialize the startup.
            nc.sync.dma_start(m_sb[:], m_r)
            nc.sync.dma_start(u_sb[:], ub.ap())
            nc.sync.dma_start(wv_sb[:], wv_r)
            nc.sync.dma_start(bv_sb[:], bvr.ap())
            nc.gpsimd.dma_start(qraw[:, :, :QC], qT_r[:, :, :QC])
            KCH = LK // 8
            for c in range(8):
                sl = slice(c * KCH, (c + 1) * KCH)
                nc.gpsimd.dma_start(kraw[:, :, sl], kT_r[:, :, sl])
                nc.gpsimd.dma_start(vraw[:, :, sl], vT_r[:, :, sl])
                if c < NQC - 1:
                    qs = slice((c + 1) * QC, (c + 2) * QC)
                    nc.gpsimd.dma_start(qraw[:, :, qs], qT_r[:, :, qs])
            nc.vector.memset(V_sb[:, :, H:HA], 1.0)

            with (
                tc.tile_pool(name="pproj", bufs=2, space="PSUM") as pp,
                tc.tile_pool(name="pt", bufs=10) as ptp,
                tc.tile_pool(name="ps_s", bufs=2, space="PSUM") as pss,
                tc.tile_pool(name="ps_ctx", bufs=4, space="PSUM") as psc,
                tc.tile_pool(name="fin", bufs=4) as fin,
            ):
                # (qM)^T projection chunk: lhsT = M[h, h~-window], rhs = qraw
                def qm_chunk(c):
                    for ot in range(HO):
                        ps = pp.tile([P, QC], f32, tag="pp")
                        for ho in range(HO):
                            nc.tensor.matmul(
                                ps[:],
                                m_sb[:, ho, ot * P:(ot + 1) * P],
                                qraw[:, ho, c * QC:(c + 1) * QC],
                                start=(ho == 0),
                                stop=(ho == HO - 1),
                            )
                        nc.vector.tensor_copy(
                            QMT[:, ot, c * QC:(c + 1) * QC], ps[:]
                        )

                # V projection chunk: V[s, o] = vraw-tile.T @ Wv^T + bv
                def v_chunk(st):
                    ps_full = pp.tile([P, QC], f32, tag="pp")
                    ps = ps_full[:, :H]
                    for ho in range(HO):
                        nc.tensor.matmul(
                            ps[:],
                            vraw[:, ho, st * P:(st + 1) * P],
                            wv_sb[:, ho, :],
                            start=(ho == 0),
                            stop=(ho == HO - 1),
                        )
                    nc.vector.tensor_tensor(V_sb[:, st, :H], ps[:], bv_sb[:], Add)

                def scores_tile(qc, kt, pts):
                    ps = pss.tile([P, QC], f32, tag="ps_s")
                    for ho in range(HO):
                        nc.tensor.matmul(
                            ps[:],
                            kraw[:, ho, kt * P:(kt + 1) * P],
                            QMT[:, ho, qc * QC:(qc + 1) * QC],
                            start=(ho == 0),
                            stop=(ho == HO - 1),
                        )
                    pt = ptp.tile([P, QC], bf16, tag="pt")
                    nc.scalar.activation(
                        pt[:], ps[:], Exp,
                        bias=u_sb[:, kt:kt + 1], scale=float(SCALE),
                    )
                    pts[kt] = pt

                def pv_step(ctx, kt, pts):
                    for qw in range(QW):
                        nc.tensor.matmul(
                            ctx[qw][:],
                            pts[kt][:, qw * P:(qw + 1) * P],
                            V_sb[:, kt, :],
                            start=(kt == 0),
                            stop=(kt == KT - 1),
                        )

                qm_chunk(0)
                for qc in range(NQC):
                    ctx = [psc.tile([P, HA], f32, tag="ps_ctx",
                                    name=f"ctx_{qc}_{qw}")
                           for qw in range(QW)]
                    pts = {}
                    for kt in range(KT):
                        scores_tile(qc, kt, pts)
                        if qc == 0:
                            # fill the first chunk's exp-bound phase with
                            # the V projection and remaining qM chunks
                            v_chunk(kt)
                            if kt % 12 == 4 and 1 + kt // 12 < NQC:
                                qm_chunk(1 + kt // 12)
                        if kt >= LAG:
                            pv_step(ctx, kt - LAG, pts)
                    for kt in range(KT - LAG, KT):
                        pv_step(ctx, kt, pts)
                    for qw in range(QW):
                        rec = fin.tile([P, 1], f32, tag="rec")
                        nc.vector.reciprocal(rec[:], ctx[qw][:, H:HA])
                        osb = fin.tile([P, H], f32, tag="osb")
                        nc.vector.tensor_scalar_mul(
                            osb[:], ctx[qw][:, :H], rec[:])
                        nc.sync.dma_start(
                            out.ap()[qc * QC + qw * P:qc * QC + (qw + 1) * P, :],
                            osb[:],
                        )
    nc.compile()
    return nc


def _get_nc():
    global _NC_CACHE
    if _NC_CACHE is None:
        _NC_CACHE = _build_nc()
    return _NC_CACHE


def _prep_in_maps(q, k, v, Wq, bq, Wk, bk, Wv, bv):
    q = np.asarray(q, np.float32)
    k = np.asarray(k, np.float32)
    v = np.asarray(v, np.float32)
    Wq = np.asarray(Wq, np.float64)
    Wk = np.asarray(Wk, np.float64)
    bq_ = np.asarray(bq, np.float64)
    bk_ = np.asarray(bk, np.float64)
    M = Wq.T @ Wk                       # [h, h~]
    w2v = Wk.T @ bq_                    # [h]
    ccv = float(bq_ @ bk_)
    mT = np.ascontiguousarray(M).astype(_BF16)          # [h, h~] == lhsT layout
    wvT = np.ascontiguousarray(np.asarray(Wv, np.float32).T).astype(_BF16)
    bvr = np.ascontiguousarray(
        np.broadcast_to(np.asarray(bv, np.float32), (P, H)))
    in_maps = []
    for i in range(NCORES):
        b, half = divmod(i, NCORES // B)
        qT_i = np.ascontiguousarray(q[b, half * NQ:(half + 1) * NQ, :].T).astype(_BF16)
        kT_i = np.ascontiguousarray(k[b].T).astype(_BF16)
        vT_i = np.ascontiguousarray(v[b].T).astype(_BF16)
        # u_k = (k.(Wk.T bq) + bq.bk)/sqrt(H), [k] -> [p, kt] with k=kt*128+p
        u = (k[b].astype(np.float64) @ w2v + ccv) * float(SCALE)
        ub_i = np.ascontiguousarray(u.reshape(KT, P).T.astype(np.float32))
        in_maps.append({
            "qT": qT_i, "kT": kT_i, "vT": vT_i,
            "mT": mT, "wvT": wvT, "ub": ub_i, "bvr": bvr,
        })
    return in_maps


def _install_ntff_hook_shim():
    """The image's antenv lacks axon_hooks; recreate it from the boot recipe
    (ctypes into libaxon_pjrt.so) so trace=True can capture NTFF profiles."""
    import types
    import contextlib
    import ctypes

    if "antenv.axon_hooks" in sys.modules:
        return
    so_path = "/opt/axon/libaxon_pjrt.so"
    hook = None
    if os.path.exists(so_path):
        lib = ctypes.CDLL(so_path)
        if hasattr(lib, "axon_start_nrt_profile"):
            lib.axon_start_nrt_profile.argtypes = [
                ctypes.POINTER(ctypes.c_int64), ctypes.c_size_t]
            lib.axon_start_nrt_profile.restype = ctypes.c_int64
            lib.axon_stop_nrt_profile.argtypes = [ctypes.c_char_p]
            lib.axon_stop_nrt_profile.restype = ctypes.c_int64

            @contextlib.contextmanager
            def _hook(output_dir, device_ids):
                import jax
                jax.devices()
                if device_ids:
                    ids = (ctypes.c_int64 * len(device_ids))(*device_ids)
                    rc = lib.axon_start_nrt_profile(ids, len(device_ids))
                else:
                    rc = lib.axon_start_nrt_profile(None, 0)
                if rc != 0:
                    raise RuntimeError(f"axon_start_nrt_profile rc={rc}")
                try:
                    yield
                finally:
                    n = lib.axon_stop_nrt_profile(str(output_dir).encode())
                    print(f"profile: {n} file(s) written to {output_dir}")

            hook = _hook
    mod = types.ModuleType("antenv.axon_hooks")
    mod.get_axon_ntff_profile_hook = lambda: hook
    mod.set_axon_ntff_profile_hook = lambda h: None
    sys.modules["antenv.axon_hooks"] = mod


def run(inputs, trace=False, trace_cores=None):
    """Run on 8 NeuronCores. Returns (output, BassKernelResults)."""
    from concourse.bass_utils import run_bass_kernel_spmd

    if trace:
        _install_ntff_hook_shim()
    nc = _get_nc()
    in_maps = _prep_in_maps(**inputs)
    res = run_bass_kernel_spmd(
        nc, in_maps, core_ids=list(range(NCORES)),
        trace=trace, trace_cores=trace_cores,
    )
    full = np.empty((B, LQ, H), np.float32)
    for i in range(NCORES):
        b, half = divmod(i, NCORES // B)
        full[b, half * NQ:(half + 1) * NQ, :] = res.results[i]["out"]
    return full, res


def kernel(**inputs):
    return run(inputs, trace=False)[0]


# revision 19
# speedup vs baseline: 1.0666x; 1.0293x over previous
"""Cross-attention kernel for Trainium2 (8 NeuronCores, SPMD).

Problem: B=4, LQ=LK=4096, H=256
  query = q @ Wq.T + bq ; keys = k @ Wk.T + bk ; values = v @ Wv.T + bv
  out = softmax(query @ keys.T / sqrt(H)) @ values

Sharding: core i -> batch i//2, query rows (i%2)*2048 .. +2048.
K/V for the batch are replicated across the 2 cores sharing it.

Device algorithm (PE contracts over the partition dim):
  - scores are algebraically refactored:
      s[q,k] = q_q M k_k^T + t_q + u_k,  M = Wq.T @ Wk  (host-folded)
      t_q = (q Wq.T)·bk   -- constant per softmax row: cancels, dropped
      u_k = (k·(Wk.T bq) + bq·bk)/sqrt(H) -- per-key scalar, computed on
            host during input prep, folded into exp as per-partition bias
    so the K projection disappears and scores read RAW k^T.
  - q/k/v are fed transposed ([h, s], h on partitions); scores are
    computed transposed ([k, q]) so exp(scores) = P^T is born k-major.
  - softmax skips max-subtraction (scores/sqrt(H) stay within ~+-7 here).
  - P@V uses P^T tiles as stationary and V augmented with a ones-column
    ([k, 257]) as moving: output column 256 is the softmax denominator
    and the context lands in natural [q, h] layout. Normalization is a
    per-partition reciprocal + tensor_scalar multiply on PSUM->SBUF.
  - score and P@V matmuls are interleaved per k-tile (P@V lags 4 tiles)
    so the exp's ScalarE latency hides behind P@V work on PE; the V and
    qM projections fill the first chunk's score phase.
"""

import os
import sys

import numpy as np

sys.path.insert(0, "/opt/trn_rl_repo")

import ml_dtypes

B, LQ, LK, H = 4, 4096, 4096, 256
P = 128
HO = H // P            # 2 h-tiles
NCORES = 8
NQ = LQ * B // NCORES  # 2048 q rows per core
QC = 512               # q chunk (scores tile width)
NQC = NQ // QC         # 4
QW = QC // P           # 4 q-windows per chunk
KT = LK // P           # 32 k tiles
HA = H + 1             # V augmented with ones column
LAG = 8                # P@V lags scores by this many k-tiles
SCALE = 1.0 / np.sqrt(np.float32(H))  # 1/16

_BF16 = ml_dtypes.bfloat16

_NC_CACHE = None


def _build_nc():
    """Build the single-core Bass program (same program runs SPMD on 8 cores)."""
    import concourse.bass as bass
    import concourse.mybir as mybir
    import concourse.tile as tile
    from concourse import bacc

    f32 = mybir.dt.float32
    bf16 = mybir.dt.bfloat16

    nc = bacc.Bacc("TRN2", target_bir_lowering=False, debug=False)

    kT = nc.declare_dram_parameter("kT", [H, LK], bf16, isOutput=False)
    qT = nc.declare_dram_parameter("qT", [H, NQ], bf16, isOutput=False)
    vT = nc.declare_dram_parameter("vT", [H, LK], bf16, isOutput=False)
    mT = nc.declare_dram_parameter("mT", [H, H], bf16, isOutput=False)   # M=Wq.T@Wk
    wvT = nc.declare_dram_parameter("wvT", [H, H], bf16, isOutput=False)
    ub = nc.declare_dram_parameter("ub", [P, KT], f32, isOutput=False)   # exp bias
    bvr = nc.declare_dram_parameter("bvr", [P, H], f32, isOutput=False)
    out = nc.declare_dram_parameter("out", [NQ, H], f32, isOutput=True)

    # [h, s] -> [p, ho, s] with h = ho*128 + p
    qT_r = qT.ap().rearrange("(o p) n -> p o n", p=P)
    kT_r = kT.ap().rearrange("(o p) n -> p o n", p=P)
    vT_r = vT.ap().rearrange("(o p) n -> p o n", p=P)
    m_r = mT.ap().rearrange("(o p) n -> p o n", p=P)
    wv_r = wvT.ap().rearrange("(o p) n -> p o n", p=P)

    Exp = mybir.ActivationFunctionType.Exp
    Add = mybir.AluOpType.add

    with tile.TileContext(nc) as tc:
        with (
            tc.tile_pool(name="consts", bufs=1) as consts,
            tc.tile_pool(name="persist", bufs=1) as persist,
        ):
            m_sb = consts.tile([P, HO, H], bf16)
            wv_sb = consts.tile([P, HO, H], bf16)
            u_sb = consts.tile([P, KT], f32)
            bv_sb = consts.tile([P, H], f32)

            kraw = persist.tile([P, HO, LK], bf16)
            qraw = persist.tile([P, HO, NQ], bf16)
            vraw = persist.tile([P, HO, LK], bf16)
            QMT = persist.tile([P, HO, NQ], bf16)   # (q M)^T  [h~, q]
            V_sb = persist.tile([P, KT, HA], bf16)  # values [k, h] + ones col

            # DMA issue order = consumption order. Small weight tensors on
            # the sync engine; bulk k/q/v loads issued from the otherwise-idle
            # gpsimd engine so issue time doesn't serialize the startup.
            nc.sync.dma_start(m_sb[:], m_r)
            nc.sync.dma_start(u_sb[:], ub.ap())
            nc.gpsimd.dma_start(qraw[:, :, :QC], qT_r[:, :, :QC])
            nc.gpsimd.dma_start(wv_sb[:], wv_r)
            nc.gpsimd.dma_start(bv_sb[:], bvr.ap())
            KCH = LK // 8
            for c in range(8):
                sl = slice(c * KCH, (c + 1) * KCH)
                nc.sync.dma_start(kraw[:, :, sl], kT_r[:, :, sl])
                nc.gpsimd.dma_start(vraw[:, :, sl], vT_r[:, :, sl])
            nc.gpsimd.dma_start(qraw[:, :, QC:2 * QC], qT_r[:, :, QC:2 * QC])
            nc.gpsimd.dma_start(qraw[:, :, 2 * QC:], qT_r[:, :, 2 * QC:])
            nc.vector.memset(V_sb[:, :, H:HA], 1.0)

            with (
                tc.tile_pool(name="pt", bufs=16) as ptp,
                tc.tile_pool(name="ps_s", bufs=4, space="PSUM") as pss,
                tc.tile_pool(name="ps_ctx", bufs=4, space="PSUM") as psc,
                tc.tile_pool(name="fin", bufs=8) as fin,
            ):
                # (qM)^T projection chunk: lhsT = M[h, h~-window], rhs = qraw
                def qm_chunk(c):
                    for ot in range(HO):
                        ps = pss.tile([P, QC], f32, tag="ps_s")
                        for ho in range(HO):
                            nc.tensor.matmul(
                                ps[:],
                                m_sb[:, ho, ot * P:(ot + 1) * P],
                                qraw[:, ho, c * QC:(c + 1) * QC],
                                start=(ho == 0),
                                stop=(ho == HO - 1),
                            )
                        nc.vector.tensor_copy(
                            QMT[:, ot, c * QC:(c + 1) * QC], ps[:]
                        )

                # V projection chunk: V[s, o] = vraw-tile.T @ Wv^T + bv
                def v_chunk(st):
                    ps_full = pss.tile([P, QC], f32, tag="ps_s")
                    ps = ps_full[:, :H]
                    for ho in range(HO):
                        nc.tensor.matmul(
                            ps[:],
                            vraw[:, ho, st * P:(st + 1) * P],
                            wv_sb[:, ho, :],
                            start=(ho == 0),
                            stop=(ho == HO - 1),
                        )
                    nc.vector.tensor_tensor(V_sb[:, st, :H], ps[:], bv_sb[:], Add)

                def scores_tile(qc, kt, pts):
                    ps = pss.tile([P, QC], f32, tag="ps_s")
                    for ho in range(HO):
                        nc.tensor.matmul(
                            ps[:],
                            kraw[:, ho, kt * P:(kt + 1) * P],
                            QMT[:, ho, qc * QC:(qc + 1) * QC],
                            start=(ho == 0),
                            stop=(ho == HO - 1),
                        )
                    pt = ptp.tile([P, QC], bf16, tag="pt")
                    nc.scalar.activation(
                        pt[:], ps[:], Exp,
                        bias=u_sb[:, kt:kt + 1], scale=float(SCALE),
                    )
                    pts[kt] = pt

                def pv_step(ctx, kt, pts):
                    for qw in range(QW):
                        nc.tensor.matmul(
                            ctx[qw][:],
                            pts[kt][:, qw * P:(qw + 1) * P],
                            V_sb[:, kt, :],
                            start=(kt == 0),
                            stop=(kt == KT - 1),
                        )

                qm_chunk(0)
                for qc in range(NQC):
                    ctx = [psc.tile([P, HA], f32, tag="ps_ctx",
                                    name=f"ctx_{qc}_{qw}")
                           for qw in range(QW)]
                    pts = {}
                    for kt in range(KT):
                        scores_tile(qc, kt, pts)
                        if qc == 0:
                            # fill the first chunk's exp-bound phase with
                            # the V projection; defer the later qM chunks
                            # to chunk 1 so their q DMAs don't compete with
                            # k/v at startup
                            v_chunk(kt)
                            if kt == 4:
                                qm_chunk(1)
                        elif qc == 1 and kt in (0, 8) and 2 + kt // 8 < NQC:
                            qm_chunk(2 + kt // 8)
                        if kt >= LAG:
                            pv_step(ctx, kt - LAG, pts)
                    # drain qw-major with fused epilogue: each ctx bank's
                    # tail matmuls finish and its normalize runs while the
                    # other banks are still draining, freeing banks early.
                    for qw in range(QW):
                        for kt in range(KT - LAG, KT):
                            nc.tensor.matmul(
                                ctx[qw][:],
                                pts[kt][:, qw * P:(qw + 1) * P],
                                V_sb[:, kt, :],
                                start=False,
                                stop=(kt == KT - 1),
                            )
                        rec = fin.tile([P, 1], f32, tag="rec")
                        nc.vector.reciprocal(rec[:], ctx[qw][:, H:HA])
                        osb = fin.tile([P, H], f32, tag="osb")
                        nc.vector.tensor_scalar_mul(
                            osb[:], ctx[qw][:, :H], rec[:])
                        nc.sync.dma_start(
                            out.ap()[qc * QC + qw * P:qc * QC + (qw + 1) * P, :],
                            osb[:],
                        )
    nc.compile()
    return nc


def _get_nc():
    global _NC_CACHE
    if _NC_CACHE is None:
        _NC_CACHE = _build_nc()
    return _NC_CACHE


def _prep_in_maps(q, k, v, Wq, bq, Wk, bk, Wv, bv):
    q = np.asarray(q, np.float32)
    k = np.asarray(k, np.float32)
    v = np.asarray(v, np.float32)
    Wq = np.asarray(Wq, np.float64)
    Wk = np.asarray(Wk, np.float64)
    bq_ = np.asarray(bq, np.float64)
    bk_ = np.asarray(bk, np.float64)
    M = Wq.T @ Wk                       # [h, h~]
    w2v = Wk.T @ bq_                    # [h]
    ccv = float(bq_ @ bk_)
    mT = np.ascontiguousarray(M).astype(_BF16)          # [h, h~] == lhsT layout
    wvT = np.ascontiguousarray(np.asarray(Wv, np.float32).T).astype(_BF16)
    bvr = np.ascontiguousarray(
        np.broadcast_to(np.asarray(bv, np.float32), (P, H)))
    in_maps = []
    for i in range(NCORES):
        b, half = divmod(i, NCORES // B)
        qT_i = np.ascontiguousarray(q[b, half * NQ:(half + 1) * NQ, :].T).astype(_BF16)
        kT_i = np.ascontiguousarray(k[b].T).astype(_BF16)
        vT_i = np.ascontiguousarray(v[b].T).astype(_BF16)
        # u_k = (k.(Wk.T bq) + bq.bk)/sqrt(H), [k] -> [p, kt] with k=kt*128+p
        u = (k[b].astype(np.float64) @ w2v + ccv) * float(SCALE)
        ub_i = np.ascontiguousarray(u.reshape(KT, P).T.astype(np.float32))
        in_maps.append({
            "qT": qT_i, "kT": kT_i, "vT": vT_i,
            "mT": mT, "wvT": wvT, "ub": ub_i, "bvr": bvr,
        })
    return in_maps


def _install_ntff_hook_shim():
    """The image's antenv lacks axon_hooks; recreate it from the boot recipe
    (ctypes into libaxon_pjrt.so) so trace=True can capture NTFF profiles."""
    import types
    import contextlib
    import ctypes

    if "antenv.axon_hooks" in sys.modules:
        return
    so_path = "/opt/axon/libaxon_pjrt.so"
    hook = None
    if os.path.exists(so_path):
        lib = ctypes.CDLL(so_path)
        if hasattr(lib, "axon_start_nrt_profile"):
            lib.axon_start_nrt_profile.argtypes = [
                ctypes.POINTER(ctypes.c_int64), ctypes.c_size_t]
            lib.axon_start_nrt_profile.restype = ctypes.c_int64
            lib.axon_stop_nrt_profile.argtypes = [ctypes.c_char_p]
            lib.axon_stop_nrt_profile.restype = ctypes.c_int64

            @contextlib.contextmanager
            def _hook(output_dir, device_ids):
                import jax
                jax.devices()
                if device_ids:
                    ids = (ctypes.c_int64 * len(device_ids))(*device_ids)
                    rc = lib.axon_start_nrt_profile(ids, len(device_ids))
                else:
                    rc = lib.axon_start_nrt_profile(None, 0)
                if rc != 0:
                    raise RuntimeError(f"axon_start_nrt_profile rc={rc}")
                try:
                    yield
                finally:
                    n = lib.axon_stop_nrt_profile(str(output_dir).encode())
                    print(f"profile: {n} file(s) written to {output_dir}")

            hook = _hook
    mod = types.ModuleType("antenv.axon_hooks")
    mod.get_axon_ntff_profile_hook = lambda: hook
    mod.set_axon_ntff_profile_hook = lambda h: None
    sys.modules["antenv.axon_hooks"] = mod


def run(inputs, trace=False, trace_cores=None):
    """Run on 8 NeuronCores. Returns (output, BassKernelResults)."""
    from concourse.bass_utils import run_bass_kernel_spmd

    if trace:
        _install_ntff_hook_shim()
    nc = _get_nc()
    in_maps = _prep_in_maps(**inputs)
    res = run_bass_kernel_spmd(
        nc, in_maps, core_ids=list(range(NCORES)),
        trace=trace, trace_cores=trace_cores,
    )
    full = np.empty((B, LQ, H), np.float32)
    for i in range(NCORES):
        b, half = divmod(i, NCORES // B)
        full[b, half * NQ:(half + 1) * NQ, :] = res.results[i]["out"]
    return full, res


def kernel(**inputs):
    return run(inputs, trace=False)[0]
